# revision 13
# baseline (speedup 1.0000x reference)
"""Trainium2 Bass kernel for nn_NerfNet: NeRF-style ray compositing.

Self-contained: shards 8192 rays across 8 NeuronCores (1024 rays/core),
runs one SPMD Bass/Tile program, gathers full outputs.

Per-core layout: [128 partitions, 8 rays/partition, 128 samples].
The O(N^2) ldist pairwise term uses the sorted-midpoint prefix-sum identity:
    sum_{ij} w_i w_j |m_i - m_j| = 2 * sum_i w_i (m_i A_i - B_i),
with A = cumsum(w), B = cumsum(w*m) (inclusive; diagonal cancels exactly).
All |ray_d| scaling is folded into z once (zs = |d| * z_ext), so dists and
midpoints come out pre-scaled; depth uses unscaled z.
"""

import numpy as np

import concourse.bacc as bacc
import concourse.mybir as mybir
import concourse.tile as tile
from concourse.bass import broadcast_tensor_aps
from concourse.bass_utils import run_bass_kernel_spmd

F32 = mybir.dt.float32
Alu = mybir.AluOpType
Act = mybir.ActivationFunctionType

R = 8192          # total rays
N = 128           # samples per ray
NCORES = 8
RPC = R // NCORES  # rays per core = 1024
P = 128            # SBUF partitions
T = RPC // P       # rays per partition = 8
EPS = 1e-6
DMA_SPLIT = 4      # chunks per large DMA

_CACHE = {}


def _emit(nc):
    d_rayd = nc.dram_tensor("ray_d", [RPC, 3], F32, kind="ExternalInput").ap()
    d_zmax = nc.dram_tensor("fg_z_max", [RPC], F32, kind="ExternalInput").ap()
    d_z = nc.dram_tensor("fg_z_vals", [RPC, N], F32, kind="ExternalInput").ap()
    d_sigma = nc.dram_tensor("sigma", [RPC, N], F32, kind="ExternalInput").ap()
    d_rgb = nc.dram_tensor("rgb", [RPC, N, 3], F32, kind="ExternalInput").ap()
    d_bg = nc.dram_tensor("bg_rgb_linear", [RPC, 3], F32, kind="ExternalInput").ap()

    d_rgbmap = nc.dram_tensor("rgb_map", [RPC, 3], F32, kind="ExternalOutput").ap()
    d_w = nc.dram_tensor("fg_weights", [RPC, N], F32, kind="ExternalOutput").ap()
    d_fgrgb = nc.dram_tensor("fg_rgb_map", [RPC, 3], F32, kind="ExternalOutput").ap()
    d_depth = nc.dram_tensor("fg_depth_map", [RPC], F32, kind="ExternalOutput").ap()
    d_bgl = nc.dram_tensor("bg_lambda", [RPC], F32, kind="ExternalOutput").ap()
    d_ldist = nc.dram_tensor("fg_ldist", [RPC], F32, kind="ExternalOutput").ap()
    d_tv = nc.dram_tensor("fg_tv", [RPC], F32, kind="ExternalOutput").ap()

    with tile.TileContext(nc) as tc:
        with tc.tile_pool(name="main", bufs=1) as pool, \
             tc.tile_pool(name="scr", bufs=4) as scrpool:
            zmx = pool.tile([P, T * (N + 1)], F32, tag="zmx")
            zs = pool.tile([P, T * (N + 1)], F32, tag="zs")
            sig = pool.tile([P, T * N], F32, tag="sig")
            rgb = pool.tile([P, T * N * 3], F32, tag="rgb")
            rayd = pool.tile([P, T * 3], F32, tag="rayd")
            bg = pool.tile([P, T * 3], F32, tag="bg")
            sq = pool.tile([P, T * 3], F32, tag="sq")
            nrm2 = pool.tile([P, T], F32, tag="nrm2")
            nrm = pool.tile([P, T], F32, tag="nrm")
            mask = pool.tile([P, T * N], F32, tag="mask")
            dists = pool.tile([P, T * N], F32, tag="dists")
            m2 = pool.tile([P, T * N], F32, tag="m2")
            sd = pool.tile([P, T * N], F32, tag="sd")
            e = pool.tile([P, T * N], F32, tag="e")
            ep = pool.tile([P, T * N], F32, tag="ep")
            alpha = pool.tile([P, T * N], F32, tag="alpha")
            Tx = pool.tile([P, T * (N + 1)], F32, tag="Tx")
            w = pool.tile([P, T * N], F32, tag="w")
            zflat = pool.tile([P, T * N], F32, tag="zflat")
            prods = pool.tile([P, 4 * T * N], F32, tag="prods")
            prodsum = pool.tile([P, 4 * T], F32, tag="prodsum")
            A = pool.tile([P, T * N], F32, tag="A")
            wm2 = pool.tile([P, T * N], F32, tag="wm2")
            B2 = pool.tile([P, T * N], F32, tag="B2")
            u = pool.tile([P, T * N], F32, tag="u")
            dw = pool.tile([P, T * (N - 1)], F32, tag="dw")
            dwa = pool.tile([P, T * (N - 1)], F32, tag="dwa")
            rgbw = pool.tile([P, T * N * 3], F32, tag="rgbw")
            fgrgb = pool.tile([P, T * 3], F32, tag="fgrgb")
            rgbmap = pool.tile([P, T * 3], F32, tag="rgbmap")
            bglp = pool.tile([P, T], F32, tag="bglp")
            bgltmp = pool.tile([P, T * 3], F32, tag="bgltmp")
            depth8 = pool.tile([P, T], F32, tag="depth8")
            l1 = pool.tile([P, T], F32, tag="l1")
            l2 = pool.tile([P, T], F32, tag="l2")
            ldist8 = pool.tile([P, T], F32, tag="ldist8")
            tv8 = pool.tile([P, T], F32, tag="tv8")

            # 3D/4D views
            z3 = zmx[:, :].rearrange("p (t n) -> p t n", t=T)       # [P,T,129]
            zs3 = zs[:, :].rearrange("p (t n) -> p t n", t=T)
            sig3 = sig[:, :].rearrange("p (t n) -> p t n", t=T)     # [P,T,128]
            m23 = m2[:, :].rearrange("p (t n) -> p t n", t=T)
            d3 = dists[:, :].rearrange("p (t n) -> p t n", t=T)
            sd3 = sd[:, :].rearrange("p (t n) -> p t n", t=T)
            e3 = e[:, :].rearrange("p (t n) -> p t n", t=T)
            ep3 = ep[:, :].rearrange("p (t n) -> p t n", t=T)
            al3 = alpha[:, :].rearrange("p (t n) -> p t n", t=T)
            T3 = Tx[:, :].rearrange("p (t n) -> p t n", t=T)        # [P,T,129]
            w3 = w[:, :].rearrange("p (t n) -> p t n", t=T)
            A3 = A[:, :].rearrange("p (t n) -> p t n", t=T)
            wm3 = wm2[:, :].rearrange("p (t n) -> p t n", t=T)
            B3 = B2[:, :].rearrange("p (t n) -> p t n", t=T)
            u3 = u[:, :].rearrange("p (t n) -> p t n", t=T)
            dw3 = dw[:, :].rearrange("p (t n) -> p t n", t=T)
            dwa3 = dwa[:, :].rearrange("p (t n) -> p t n", t=T)
            m3 = mask[:, :].rearrange("p (t n) -> p t n", t=T)
            rgb4 = rgb[:, :].rearrange("p (t n c) -> p t n c", t=T, n=N, c=3)
            rgbw4 = rgbw[:, :].rearrange("p (t n c) -> p t n c", t=T, n=N, c=3)
            rgbw4r = rgbw[:, :].rearrange("p (t n c) -> p t c n", t=T, n=N, c=3)
            fgrgb3 = fgrgb[:, :].rearrange("p (t c) -> p t c", t=T)
            rgbmap3 = rgbmap[:, :].rearrange("p (t c) -> p t c", t=T)
            bg3 = bg[:, :].rearrange("p (t c) -> p t c", t=T)
            bgltmp3 = bgltmp[:, :].rearrange("p (t c) -> p t c", t=T)
            sq3 = sq[:, :].rearrange("p (t c) -> p t c", t=T)

            # ---- DMAs in (split across queues) ----
            dz = d_z.rearrange("(p t) n -> p t n", p=P)
            dsig = d_sigma.rearrange("(p t) n -> p (t n)", p=P)
            dsig_z = d_z.rearrange("(p t) n -> p (t n)", p=P)
            drgb = d_rgb.rearrange("(p t) n c -> p (t n c)", p=P)
            chunk = P // DMA_SPLIT
            nc.gpsimd.dma_start(rayd[:, :], d_rayd.rearrange("(p t) c -> p (t c)", p=P))
            for k in range(DMA_SPLIT):
                pr = slice(k * chunk, (k + 1) * chunk)
                nc.gpsimd.dma_start(z3[pr, :, 0:N], dz[pr, :, :])
            nc.gpsimd.dma_start(z3[:, :, N], d_zmax.rearrange("(p t) -> p t", p=P))
            for k in range(DMA_SPLIT):
                pr = slice(k * chunk, (k + 1) * chunk)
                nc.gpsimd.dma_start(zflat[pr, :], dsig_z[pr, :])
            for k in range(DMA_SPLIT):
                pr = slice(k * chunk, (k + 1) * chunk)
                nc.sync.dma_start(sig[pr, :], dsig[pr, :])
            nc.sync.dma_start(bg[:, :], d_bg.rearrange("(p t) c -> p (t c)", p=P))
            for k in range(DMA_SPLIT):
                pr = slice(k * chunk, (k + 1) * chunk)
                nc.sync.dma_start(rgb[pr, :], drgb[pr, :])

            # ---- constants ----
            nc.gpsimd.memset(mask[:, :], 1.0)
            nc.gpsimd.memset(m3[:, :, 0], 0.0)
            nc.vector.memset(T3[:, :, 0], 1.0)

            # ---- ray norm ----
            nc.vector.tensor_tensor(sq[:, :], rayd[:, :], rayd[:, :], op=Alu.mult)
            nc.vector.tensor_reduce(nrm2[:, :], sq3, axis=mybir.AxisListType.X,
                                    op=Alu.add)
            nc.scalar.activation(nrm[:, :], nrm2[:, :], Act.Sqrt)

            # zs = |d| * z_ext  (per-ray scale via ScalarE slice ops)
            for t in range(T):
                nc.scalar.activation(zs3[:, t, :], z3[:, t, :], Act.Copy,
                                     scale=nrm[:, t:t + 1])

            # dists / midpoint-sums (scaled); m2 = 2*mid
            nc.vector.tensor_tensor(d3, zs3[:, :, 1:N + 1], zs3[:, :, 0:N],
                                    op=Alu.subtract)
            nc.vector.tensor_tensor(m23, zs3[:, :, 1:N + 1], zs3[:, :, 0:N],
                                    op=Alu.add)

            # alpha compositing
            nc.vector.tensor_tensor(sd[:, :], sig[:, :], dists[:, :], op=Alu.mult)
            nc.scalar.activation(e[:, :], sd[:, :], Act.Exp, scale=-1.0)
            epsb = pool.tile([P, 1], F32, tag="epsb")
            nc.vector.memset(epsb[:, :], EPS)
            nc.scalar.activation(ep[:, :], e[:, :], Act.Identity,
                                 bias=epsb[:, :])
            for t in range(T):
                nc.vector.tensor_tensor_scan(
                    T3[:, t, 1:N + 1], ep3[:, t, :], ep3[:, t, :], 1.0,
                    op0=Alu.mult, op1=Alu.bypass)
            nc.scalar.activation(alpha[:, :], e[:, :], Act.Identity,
                                 bias=1.0, scale=-1.0)
            for t in range(T):
                nc.vector.tensor_tensor(w3[:, t, :], al3[:, t, :],
                                        T3[:, t, 0:N], op=Alu.mult)

            dwout_t = d_w.rearrange("(p t) n -> p (t n)", p=P)
            for k in range(DMA_SPLIT):
                pr = slice(k * chunk, (k + 1) * chunk)
                nc.scalar.dma_start(dwout_t[pr, :], w[pr, :])
            # tv = sum |diff(w)|
            wneg = pool.tile([P, T * N], F32, tag="wneg")
            nc.scalar.activation(wneg[:, :], w[:, :], Act.Copy, scale=-1.0)
            wneg3 = wneg[:, :].rearrange("p (t n) -> p t n", t=T)
            nc.gpsimd.dma_start(dw3, wneg3[:, :, 0:N - 1])
            nc.gpsimd.dma_start(dw3, w3[:, :, 1:N], accum_op=Alu.add)
            for t in range(T):
                nc.scalar.activation(dwa3[:, t, :], dw3[:, t, :], Act.Abs,
                                     accum_out=tv8[:, t:t + 1])

            # rgb: fgrgb[t,c] = sum_n w*rgb
            w4 = w[:, :].rearrange("p (t n one) -> p t n one", t=T, one=1)
            a_rgb, a_w = broadcast_tensor_aps(rgb4, w4)
            nc.vector.tensor_tensor(rgbw4, a_rgb, a_w, op=Alu.mult)
            nc.vector.tensor_reduce(fgrgb3, rgbw4r, axis=mybir.AxisListType.X,
                                    op=Alu.add)

            # rgb_map = fgrgb + bg_lambda * bg
            nc.vector.tensor_copy(bglp[:, :], T3[:, :, N])
            bglp3 = bglp[:, :].rearrange("p (t one) -> p t one", one=1)
            a_bg, a_bgl = broadcast_tensor_aps(bg3, bglp3)
            nc.vector.tensor_tensor(bgltmp3, a_bg, a_bgl, op=Alu.mult)
            nc.vector.tensor_tensor(rgbmap[:, :], fgrgb[:, :], bgltmp[:, :],
                                    op=Alu.add)

            # prefix sums for ldist
            nc.vector.tensor_tensor_scan(A[:, :], mask[:, :], w[:, :], 0.0,
                                         op0=Alu.mult, op1=Alu.add)
            nc.vector.tensor_tensor(wm2[:, :], w[:, :], m2[:, :], op=Alu.mult)
            nc.vector.tensor_tensor_scan(B2[:, :], mask[:, :], wm2[:, :], 0.0,
                                         op0=Alu.mult, op1=Alu.add)

            # w^2 for the square term
            nc.scalar.activation(u[:, :], w[:, :], Act.Square)

            # per-ray reductions: four flat products into one tile, then a
            # single grouped 4D reduce over the samples axis.
            # ldist = sum(wm2*A) - sum(w*B2) + (1/3)*sum(w^2*dists)
            #   [m2 = 2*mid and B2 = 2*B cancel the pair-term factor 2]
            # depth = sum(w*zflat)
            TN = T * N
            nc.vector.tensor_tensor(prods[:, 0:TN], w[:, :], zflat[:, :],
                                    op=Alu.mult)
            nc.vector.tensor_tensor(prods[:, TN:2 * TN], wm2[:, :], A[:, :],
                                    op=Alu.mult)
            nc.vector.tensor_tensor(prods[:, 2 * TN:3 * TN], w[:, :], B2[:, :],
                                    op=Alu.mult)
            nc.vector.tensor_tensor(prods[:, 3 * TN:4 * TN], u[:, :],
                                    dists[:, :], op=Alu.mult)
            prods4 = prods[:, :].rearrange("p (g t n) -> p g t n", g=4, t=T)
            psum3 = prodsum[:, :].rearrange("p (g t) -> p g t", g=4)
            nc.vector.tensor_reduce(psum3, prods4, axis=mybir.AxisListType.X,
                                    op=Alu.add)
            depth8v = prodsum[:, 0:T]
            l1v = prodsum[:, T:2 * T]
            l2v = prodsum[:, 2 * T:3 * T]
            l3v = prodsum[:, 3 * T:4 * T]
            nc.vector.tensor_tensor(l2[:, :], l1v, l2v, op=Alu.subtract)
            nc.vector.scalar_tensor_tensor(
                ldist8[:, :], l3v, 1.0 / 3.0, l2[:, :],
                op0=Alu.mult, op1=Alu.add)

            # ---- remaining DMAs out ----
            nc.sync.dma_start(d_rgbmap.rearrange("(p t) c -> p (t c)", p=P),
                              rgbmap[:, :])
            nc.sync.dma_start(d_fgrgb.rearrange("(p t) c -> p (t c)", p=P),
                              fgrgb[:, :])
            nc.sync.dma_start(d_depth.rearrange("(p t) -> p t", p=P), depth8v)
            nc.sync.dma_start(d_bgl.rearrange("(p t) -> p t", p=P), bglp[:, :])
            nc.sync.dma_start(d_ldist.rearrange("(p t) -> p t", p=P), ldist8[:, :])
            nc.sync.dma_start(d_tv.rearrange("(p t) -> p t", p=P), tv8[:, :])


def build():
    if "nc" not in _CACHE:
        nc = bacc.Bacc("TRN2", target_bir_lowering=False, debug=False,
                       enable_asserts=False)
        _emit(nc)
        nc.compile()
        _CACHE["nc"] = nc
    return _CACHE["nc"]


def make_in_maps(ray_d, fg_z_max, fg_z_vals, sigma, rgb, bg_rgb_linear):
    ins = {
        "ray_d": np.ascontiguousarray(ray_d, np.float32),
        "fg_z_max": np.ascontiguousarray(fg_z_max, np.float32),
        "fg_z_vals": np.ascontiguousarray(fg_z_vals, np.float32),
        "sigma": np.ascontiguousarray(sigma, np.float32),
        "rgb": np.ascontiguousarray(rgb, np.float32),
        "bg_rgb_linear": np.ascontiguousarray(bg_rgb_linear, np.float32),
    }
    return [{k: v[i * RPC:(i + 1) * RPC] for k, v in ins.items()}
            for i in range(NCORES)]


def gather(res):
    def cat(name):
        return np.concatenate([res[i][name] for i in range(NCORES)], axis=0)

    return (cat("rgb_map"), cat("fg_weights"), cat("fg_rgb_map"),
            cat("fg_depth_map"), cat("bg_lambda"), cat("fg_ldist"),
            cat("fg_tv"))


def kernel(ray_d, fg_z_max, fg_z_vals, sigma, rgb, bg_rgb_linear):
    assert ray_d.shape == (R, 3) and fg_z_vals.shape == (R, N)
    nc = build()
    in_maps = make_in_maps(ray_d, fg_z_max, fg_z_vals, sigma, rgb,
                           bg_rgb_linear)
    res = run_bass_kernel_spmd(nc, in_maps, list(range(NCORES))).results
    return gather(res)


# revision 14
# speedup vs baseline: 1.1341x; 1.1341x over previous
"""Trainium2 Bass kernel for nn_NerfNet: NeRF-style ray compositing.

Self-contained: shards 8192 rays across 8 NeuronCores (1024 rays/core),
runs one SPMD Bass/Tile program, gathers full outputs.

Per-core layout: [128 partitions, 8 rays/partition, 128 samples].
The O(N^2) ldist pairwise term uses the sorted-midpoint prefix-sum identity:
    sum_{ij} w_i w_j |m_i - m_j| = 2 * sum_i w_i (m_i A_i - B_i),
with A = cumsum(w), B = cumsum(w*m) (inclusive; diagonal cancels exactly).
All |ray_d| scaling is folded into z once (zs = |d| * z_ext), so dists and
midpoints come out pre-scaled; depth uses unscaled z.
"""

import numpy as np

import concourse.bacc as bacc
import concourse.mybir as mybir
import concourse.tile as tile
from concourse.bass import broadcast_tensor_aps
from concourse.bass_utils import run_bass_kernel_spmd

F32 = mybir.dt.float32
Alu = mybir.AluOpType
Act = mybir.ActivationFunctionType

R = 8192          # total rays
N = 128           # samples per ray
NCORES = 8
RPC = R // NCORES  # rays per core = 1024
P = 128            # SBUF partitions
T = RPC // P       # rays per partition = 8
EPS = 1e-6
DMA_SPLIT = 4      # chunks per large DMA

_CACHE = {}


def _emit(nc):
    d_rayd = nc.dram_tensor("ray_d", [RPC, 3], F32, kind="ExternalInput").ap()
    d_zmax = nc.dram_tensor("fg_z_max", [RPC], F32, kind="ExternalInput").ap()
    d_z = nc.dram_tensor("fg_z_vals", [RPC, N], F32, kind="ExternalInput").ap()
    d_sigma = nc.dram_tensor("sigma", [RPC, N], F32, kind="ExternalInput").ap()
    d_rgb = nc.dram_tensor("rgb", [RPC, N, 3], F32, kind="ExternalInput").ap()
    d_bg = nc.dram_tensor("bg_rgb_linear", [RPC, 3], F32, kind="ExternalInput").ap()

    d_rgbmap = nc.dram_tensor("rgb_map", [RPC, 3], F32, kind="ExternalOutput").ap()
    d_w = nc.dram_tensor("fg_weights", [RPC, N], F32, kind="ExternalOutput").ap()
    d_fgrgb = nc.dram_tensor("fg_rgb_map", [RPC, 3], F32, kind="ExternalOutput").ap()
    d_depth = nc.dram_tensor("fg_depth_map", [RPC], F32, kind="ExternalOutput").ap()
    d_bgl = nc.dram_tensor("bg_lambda", [RPC], F32, kind="ExternalOutput").ap()
    d_ldist = nc.dram_tensor("fg_ldist", [RPC], F32, kind="ExternalOutput").ap()
    d_tv = nc.dram_tensor("fg_tv", [RPC], F32, kind="ExternalOutput").ap()

    with tile.TileContext(nc) as tc:
        with tc.tile_pool(name="main", bufs=1) as pool, \
             tc.tile_pool(name="scr", bufs=4) as scrpool:
            zmx = pool.tile([P, T * (N + 1)], F32, tag="zmx")
            zs = pool.tile([P, T * (N + 1)], F32, tag="zs")
            sig = pool.tile([P, T * N], F32, tag="sig")
            rgb = pool.tile([P, T * N * 3], F32, tag="rgb")
            rayd = pool.tile([P, T * 3], F32, tag="rayd")
            bg = pool.tile([P, T * 3], F32, tag="bg")
            sq = pool.tile([P, T * 3], F32, tag="sq")
            nrm2 = pool.tile([P, T], F32, tag="nrm2")
            nrm = pool.tile([P, T], F32, tag="nrm")
            mask = pool.tile([P, T * N], F32, tag="mask")
            dists = pool.tile([P, T * N], F32, tag="dists")
            m2 = pool.tile([P, T * N], F32, tag="m2")
            sd = pool.tile([P, T * N], F32, tag="sd")
            e = pool.tile([P, T * N], F32, tag="e")
            ep = pool.tile([P, T * N], F32, tag="ep")
            alpha = pool.tile([P, T * N], F32, tag="alpha")
            Tx = pool.tile([P, T * (N + 1)], F32, tag="Tx")
            w = pool.tile([P, T * N], F32, tag="w")
            zflat = pool.tile([P, T * N], F32, tag="zflat")
            prods = pool.tile([P, 4 * T * N], F32, tag="prods")
            prodsum = pool.tile([P, 4 * T], F32, tag="prodsum")
            A = pool.tile([P, T * N], F32, tag="A")
            wm2 = pool.tile([P, T * N], F32, tag="wm2")
            B2 = pool.tile([P, T * N], F32, tag="B2")
            u = pool.tile([P, T * N], F32, tag="u")
            dw = pool.tile([P, T * (N - 1)], F32, tag="dw")
            dwa = pool.tile([P, T * (N - 1)], F32, tag="dwa")
            rgbw = pool.tile([P, T * N * 3], F32, tag="rgbw")
            fgrgb = pool.tile([P, T * 3], F32, tag="fgrgb")
            rgbmap = pool.tile([P, T * 3], F32, tag="rgbmap")
            bglp = pool.tile([P, T], F32, tag="bglp")
            bgltmp = pool.tile([P, T * 3], F32, tag="bgltmp")
            depth8 = pool.tile([P, T], F32, tag="depth8")
            l1 = pool.tile([P, T], F32, tag="l1")
            l2 = pool.tile([P, T], F32, tag="l2")
            ldist8 = pool.tile([P, T], F32, tag="ldist8")
            tv8 = pool.tile([P, T], F32, tag="tv8")

            # 3D/4D views
            z3 = zmx[:, :].rearrange("p (t n) -> p t n", t=T)       # [P,T,129]
            zs3 = zs[:, :].rearrange("p (t n) -> p t n", t=T)
            sig3 = sig[:, :].rearrange("p (t n) -> p t n", t=T)     # [P,T,128]
            m23 = m2[:, :].rearrange("p (t n) -> p t n", t=T)
            d3 = dists[:, :].rearrange("p (t n) -> p t n", t=T)
            sd3 = sd[:, :].rearrange("p (t n) -> p t n", t=T)
            e3 = e[:, :].rearrange("p (t n) -> p t n", t=T)
            ep3 = ep[:, :].rearrange("p (t n) -> p t n", t=T)
            al3 = alpha[:, :].rearrange("p (t n) -> p t n", t=T)
            T3 = Tx[:, :].rearrange("p (t n) -> p t n", t=T)        # [P,T,129]
            w3 = w[:, :].rearrange("p (t n) -> p t n", t=T)
            A3 = A[:, :].rearrange("p (t n) -> p t n", t=T)
            wm3 = wm2[:, :].rearrange("p (t n) -> p t n", t=T)
            B3 = B2[:, :].rearrange("p (t n) -> p t n", t=T)
            u3 = u[:, :].rearrange("p (t n) -> p t n", t=T)
            dw3 = dw[:, :].rearrange("p (t n) -> p t n", t=T)
            dwa3 = dwa[:, :].rearrange("p (t n) -> p t n", t=T)
            m3 = mask[:, :].rearrange("p (t n) -> p t n", t=T)
            rgb4 = rgb[:, :].rearrange("p (t n c) -> p t n c", t=T, n=N, c=3)
            rgbw4 = rgbw[:, :].rearrange("p (t n c) -> p t n c", t=T, n=N, c=3)
            rgbw4r = rgbw[:, :].rearrange("p (t n c) -> p t c n", t=T, n=N, c=3)
            fgrgb3 = fgrgb[:, :].rearrange("p (t c) -> p t c", t=T)
            rgbmap3 = rgbmap[:, :].rearrange("p (t c) -> p t c", t=T)
            bg3 = bg[:, :].rearrange("p (t c) -> p t c", t=T)
            bgltmp3 = bgltmp[:, :].rearrange("p (t c) -> p t c", t=T)
            sq3 = sq[:, :].rearrange("p (t c) -> p t c", t=T)

            # ---- DMAs in (split across queues) ----
            dz = d_z.rearrange("(p t) n -> p t n", p=P)
            dsig = d_sigma.rearrange("(p t) n -> p (t n)", p=P)
            dsig_z = d_z.rearrange("(p t) n -> p (t n)", p=P)
            drgb = d_rgb.rearrange("(p t) n c -> p (t n c)", p=P)
            chunk = P // DMA_SPLIT
            nc.sync.dma_start(rayd[:, :], d_rayd.rearrange("(p t) c -> p (t c)", p=P))
            for k in range(DMA_SPLIT):
                pr = slice(k * chunk, (k + 1) * chunk)
                nc.sync.dma_start(z3[pr, :, 0:N], dz[pr, :, :])
            nc.sync.dma_start(z3[:, :, N], d_zmax.rearrange("(p t) -> p t", p=P))
            for k in range(DMA_SPLIT):
                pr = slice(k * chunk, (k + 1) * chunk)
                nc.sync.dma_start(zflat[pr, :], dsig_z[pr, :])
            for k in range(DMA_SPLIT):
                pr = slice(k * chunk, (k + 1) * chunk)
                nc.sync.dma_start(sig[pr, :], dsig[pr, :])
            nc.sync.dma_start(bg[:, :], d_bg.rearrange("(p t) c -> p (t c)", p=P))
            for k in range(DMA_SPLIT):
                pr = slice(k * chunk, (k + 1) * chunk)
                nc.sync.dma_start(rgb[pr, :], drgb[pr, :])

            # ---- constants ----
            nc.gpsimd.memset(mask[:, :], 1.0)
            nc.gpsimd.memset(m3[:, :, 0], 0.0)
            nc.vector.memset(T3[:, :, 0], 1.0)

            # ---- ray norm ----
            nc.vector.tensor_tensor(sq[:, :], rayd[:, :], rayd[:, :], op=Alu.mult)
            nc.vector.tensor_reduce(nrm2[:, :], sq3, axis=mybir.AxisListType.X,
                                    op=Alu.add)
            nc.scalar.activation(nrm[:, :], nrm2[:, :], Act.Sqrt)

            # zs = |d| * z_ext  (per-ray scale via ScalarE slice ops)
            for t in range(T):
                nc.scalar.activation(zs3[:, t, :], z3[:, t, :], Act.Copy,
                                     scale=nrm[:, t:t + 1])

            # dists / midpoint-sums (scaled); m2 = 2*mid
            nc.vector.tensor_tensor(d3, zs3[:, :, 1:N + 1], zs3[:, :, 0:N],
                                    op=Alu.subtract)
            nc.vector.tensor_tensor(m23, zs3[:, :, 1:N + 1], zs3[:, :, 0:N],
                                    op=Alu.add)

            # alpha compositing
            nc.vector.tensor_tensor(sd[:, :], sig[:, :], dists[:, :], op=Alu.mult)
            nc.scalar.activation(e[:, :], sd[:, :], Act.Exp, scale=-1.0)
            epsb = pool.tile([P, 1], F32, tag="epsb")
            nc.vector.memset(epsb[:, :], EPS)
            nc.scalar.activation(ep[:, :], e[:, :], Act.Identity,
                                 bias=epsb[:, :])
            for t in range(T):
                nc.vector.tensor_tensor_scan(
                    T3[:, t, 1:N + 1], ep3[:, t, :], ep3[:, t, :], 1.0,
                    op0=Alu.mult, op1=Alu.bypass)
            nc.scalar.activation(alpha[:, :], e[:, :], Act.Identity,
                                 bias=1.0, scale=-1.0)
            for t in range(T):
                nc.vector.tensor_tensor(w3[:, t, :], al3[:, t, :],
                                        T3[:, t, 0:N], op=Alu.mult)

            dwout_t = d_w.rearrange("(p t) n -> p (t n)", p=P)
            for k in range(DMA_SPLIT):
                pr = slice(k * chunk, (k + 1) * chunk)
                nc.sync.dma_start(dwout_t[pr, :], w[pr, :])
            # tv = sum |diff(w)|
            wneg = pool.tile([P, T * N], F32, tag="wneg")
            nc.scalar.activation(wneg[:, :], w[:, :], Act.Copy, scale=-1.0)
            wneg3 = wneg[:, :].rearrange("p (t n) -> p t n", t=T)
            nc.gpsimd.dma_start(dw3, wneg3[:, :, 0:N - 1])
            nc.gpsimd.dma_start(dw3, w3[:, :, 1:N], accum_op=Alu.add)
            for t in range(T):
                nc.scalar.activation(dwa3[:, t, :], dw3[:, t, :], Act.Abs,
                                     accum_out=tv8[:, t:t + 1])

            # rgb: fgrgb[t,c] = sum_n w*rgb
            w4 = w[:, :].rearrange("p (t n one) -> p t n one", t=T, one=1)
            a_rgb, a_w = broadcast_tensor_aps(rgb4, w4)
            nc.vector.tensor_tensor(rgbw4, a_rgb, a_w, op=Alu.mult)
            nc.vector.tensor_reduce(fgrgb3, rgbw4r, axis=mybir.AxisListType.X,
                                    op=Alu.add)

            # rgb_map = fgrgb + bg_lambda * bg
            nc.vector.tensor_copy(bglp[:, :], T3[:, :, N])
            bglp3 = bglp[:, :].rearrange("p (t one) -> p t one", one=1)
            a_bg, a_bgl = broadcast_tensor_aps(bg3, bglp3)
            nc.vector.tensor_tensor(bgltmp3, a_bg, a_bgl, op=Alu.mult)
            nc.vector.tensor_tensor(rgbmap[:, :], fgrgb[:, :], bgltmp[:, :],
                                    op=Alu.add)

            # prefix sums for ldist
            nc.vector.tensor_tensor_scan(A[:, :], mask[:, :], w[:, :], 0.0,
                                         op0=Alu.mult, op1=Alu.add)
            nc.vector.tensor_tensor(wm2[:, :], w[:, :], m2[:, :], op=Alu.mult)
            nc.vector.tensor_tensor_scan(B2[:, :], mask[:, :], wm2[:, :], 0.0,
                                         op0=Alu.mult, op1=Alu.add)

            # w^2 for the square term
            nc.scalar.activation(u[:, :], w[:, :], Act.Square)

            # per-ray reductions: four flat products into one tile, then a
            # single grouped 4D reduce over the samples axis.
            # ldist = sum(wm2*A) - sum(w*B2) + (1/3)*sum(w^2*dists)
            #   [m2 = 2*mid and B2 = 2*B cancel the pair-term factor 2]
            # depth = sum(w*zflat)
            TN = T * N
            nc.vector.tensor_tensor(prods[:, 0:TN], w[:, :], zflat[:, :],
                                    op=Alu.mult)
            nc.vector.tensor_tensor(prods[:, TN:2 * TN], wm2[:, :], A[:, :],
                                    op=Alu.mult)
            nc.vector.tensor_tensor(prods[:, 2 * TN:3 * TN], w[:, :], B2[:, :],
                                    op=Alu.mult)
            nc.vector.tensor_tensor(prods[:, 3 * TN:4 * TN], u[:, :],
                                    dists[:, :], op=Alu.mult)
            prods4 = prods[:, :].rearrange("p (g t n) -> p g t n", g=4, t=T)
            psum3 = prodsum[:, :].rearrange("p (g t) -> p g t", g=4)
            nc.vector.tensor_reduce(psum3, prods4, axis=mybir.AxisListType.X,
                                    op=Alu.add)
            depth8v = prodsum[:, 0:T]
            l1v = prodsum[:, T:2 * T]
            l2v = prodsum[:, 2 * T:3 * T]
            l3v = prodsum[:, 3 * T:4 * T]
            nc.vector.tensor_tensor(l2[:, :], l1v, l2v, op=Alu.subtract)
            nc.vector.scalar_tensor_tensor(
                ldist8[:, :], l3v, 1.0 / 3.0, l2[:, :],
                op0=Alu.mult, op1=Alu.add)

            # ---- remaining DMAs out ----
            nc.sync.dma_start(d_rgbmap.rearrange("(p t) c -> p (t c)", p=P),
                              rgbmap[:, :])
            nc.sync.dma_start(d_fgrgb.rearrange("(p t) c -> p (t c)", p=P),
                              fgrgb[:, :])
            nc.sync.dma_start(d_depth.rearrange("(p t) -> p t", p=P), depth8v)
            nc.sync.dma_start(d_bgl.rearrange("(p t) -> p t", p=P), bglp[:, :])
            nc.sync.dma_start(d_ldist.rearrange("(p t) -> p t", p=P), ldist8[:, :])
            nc.sync.dma_start(d_tv.rearrange("(p t) -> p t", p=P), tv8[:, :])


def build():
    if "nc" not in _CACHE:
        nc = bacc.Bacc("TRN2", target_bir_lowering=False, debug=False,
                       enable_asserts=False)
        _emit(nc)
        nc.compile()
        _CACHE["nc"] = nc
    return _CACHE["nc"]


def make_in_maps(ray_d, fg_z_max, fg_z_vals, sigma, rgb, bg_rgb_linear):
    ins = {
        "ray_d": np.ascontiguousarray(ray_d, np.float32),
        "fg_z_max": np.ascontiguousarray(fg_z_max, np.float32),
        "fg_z_vals": np.ascontiguousarray(fg_z_vals, np.float32),
        "sigma": np.ascontiguousarray(sigma, np.float32),
        "rgb": np.ascontiguousarray(rgb, np.float32),
        "bg_rgb_linear": np.ascontiguousarray(bg_rgb_linear, np.float32),
    }
    return [{k: v[i * RPC:(i + 1) * RPC] for k, v in ins.items()}
            for i in range(NCORES)]


def gather(res):
    def cat(name):
        return np.concatenate([res[i][name] for i in range(NCORES)], axis=0)

    return (cat("rgb_map"), cat("fg_weights"), cat("fg_rgb_map"),
            cat("fg_depth_map"), cat("bg_lambda"), cat("fg_ldist"),
            cat("fg_tv"))


def kernel(ray_d, fg_z_max, fg_z_vals, sigma, rgb, bg_rgb_linear):
    assert ray_d.shape == (R, 3) and fg_z_vals.shape == (R, N)
    nc = build()
    in_maps = make_in_maps(ray_d, fg_z_max, fg_z_vals, sigma, rgb,
                           bg_rgb_linear)
    res = run_bass_kernel_spmd(nc, in_maps, list(range(NCORES))).results
    return gather(res)
